# revision 19
# baseline (speedup 1.0000x reference)
"""CFG dual self-attention kernel for 8 Trainium2 NeuronCores.

Strategy (tensor parallel on heads):
  - h = concat(hidden_cond, hidden_uncond) -> [4096 tokens, 5120]; host
    pre-transposes to hT [5120, 4096] and quantizes it (and all weights) into
    fp8e4 hi/lo pairs so the big projections run as fp8 DoubleRow matmuls
    (0.5 PE cycles/row, 2 contraction chunks per pass).  Dropping the lo*lo
    cross term, each projection needs 3 DoubleRow passes per chunk-pair =
    0.75x the fp32r PE cycles at ~0.2% relative error.
  - Each core owns 5 heads (640 of the 5120 q/k/v channels).  q and k share
    one streamed pass over hT (both weight blocks resident in fp8); v runs a
    second pass.  RMSNorm over the full 5120 dims needs a cross-core sum of
    squares: per-token partials are computed with ap-size-1 ones-matmuls
    (output [tokens,1], nearly free on the PE), transposed back to row layout
    with one PE transpose, and allreduced (32 KB collective, hidden under the
    v projection).
  - Attention per (batch, head) in scores-transposed layout
    scoresT[st, sq] = (rope(k) slice)^T @ rope(q), fp32r matmuls: softmax
    denominators via ap-size-1 matmuls with et as the stationary operand
    (output [sq,1], ~free) instead of 512-cycle ones-matmuls; exp on the
    scalar engine over two score chunks at a time; A@V accumulated with
    v-chunks stationary.  The 1/colsum normalization (times the fp8 output
    scale) is applied on the eviction path, which also splits the attention
    output into fp8 hi/lo for the output projection.
  - Output projection: 3-term fp8 DoubleRow matmuls (the odd 5th contraction
    chunk gets its hi*hi and lo*lo terms fused into one DoubleRow pass via a
    swapped duplicate weight slot).  partial_out written as bf16; host sums
    the 8 partials (+ bias) in fp64.
"""

import numpy as np
import ml_dtypes

import concourse.bass as bass  # noqa: F401  (bass types via bacc)
import concourse.mybir as mybir
import concourse.tile as tile
from concourse import bacc
from concourse.bass_utils import run_bass_kernel_spmd

F32 = mybir.dt.float32
F32R = mybir.dt.float32r
F8 = mybir.dt.float8e4
BF16 = mybir.dt.bfloat16
E4NP = ml_dtypes.float8_e4m3
DRM = mybir.MatmulPerfMode.DoubleRow

NCORES = 8
EPS = 1e-6

# fp8 quantization scales (host-side hi/lo split)
SH = 16.0      # hidden states
SW = 1024.0    # qkv weights
SA = 32.0      # attention output
SWO = 1024.0   # output-projection weights
QKV_DESC = 1.0 / (SH * SW)
OUT_DESC = 1.0 / (SA * SWO)


def build_program(S, DIM, H, collective=True):
    """Emit the per-core bass program (identical on all cores; per-core data
    differences come entirely from the input tensors)."""
    HD = 128
    assert DIM == H * HD
    HPC = H // NCORES          # heads per core
    CW = HPC * HD              # per-core channel width for q/k/v
    CT = HPC                   # 128-col tiles per group
    NT = 2 * S                 # tokens across both batches
    DC = DIM // 128            # contraction chunks
    TBS = 256                  # token block in phase 1
    NTB = NT // TBS
    SQB = min(512, S)          # sq block in attention
    NSQ = S // SQB
    NST = S // 128             # st (key) chunks per batch
    ONB = DIM // 512           # out-proj N blocks
    # contraction sub-blocks for merged DMA (dep granularity)
    CSUB = 10 if DC % 10 == 0 else DC
    NCS = DC // CSUB

    nc = bacc.Bacc("TRN2", target_bir_lowering=False, debug=False,
                   num_devices=NCORES)

    # fp8 hi/lo inputs: hT8 rows (chunk, partition, j) with j=(lo, hi);
    # w*8 rows (chunk, partition, j) with j=(hi, lo)
    hT8 = nc.dram_tensor("hT8", [DIM * 2, NT], F8, kind="ExternalInput")
    wq8 = nc.dram_tensor("wq8", [DIM * 2, CW], F8, kind="ExternalInput")
    wk8 = nc.dram_tensor("wk8", [DIM * 2, CW], F8, kind="ExternalInput")
    wv8 = nc.dram_tensor("wv8", [DIM * 2, CW], F8, kind="ExternalInput")
    bq = nc.dram_tensor("bq", [128, CT], F32, kind="ExternalInput")
    bk = nc.dram_tensor("bk", [128, CT], F32, kind="ExternalInput")
    wqn = nc.dram_tensor("wqn", [128, CT], F32, kind="ExternalInput")
    wkn = nc.dram_tensor("wkn", [128, CT], F32, kind="ExternalInput")
    cosT = nc.dram_tensor("cosT", [128, S], BF16, kind="ExternalInput")
    sinrT = nc.dram_tensor("sinrT", [128, S], BF16, kind="ExternalInput")
    ident = nc.dram_tensor("ident", [128, 128], F32, kind="ExternalInput")
    # out-proj weights: rows (slot, j, partition); slots 0..4 = chunk (hi,lo),
    # slot 5 = chunk 4 again as (lo,hi) so the odd chunk's hi*hi+lo*lo terms
    # fit one DoubleRow pass
    wo8 = nc.dram_tensor("wo8", [6 * 2 * 128, DIM], F8, kind="ExternalInput")
    outp = nc.dram_tensor("outp", [NT, DIM], BF16, kind="ExternalOutput")

    h4 = hT8.rearrange("(c j p) t -> p c j t", p=128, j=2)

    with tile.TileContext(nc) as tc:
        with (
            tc.tile_pool(name="dram", bufs=1, space="DRAM") as dram,
            tc.tile_pool(name="persist", bufs=1) as persist,
        ):
            qsc = dram.tile([CW, NT], BF16, tag="qsc")
            ksc = dram.tile([CW, NT], BF16, tag="ksc")
            vsc = dram.tile([NT, CW], BF16, tag="vsc")
            aosc8 = dram.tile([2 * CW, NT], F8, tag="aosc8")
            cc_in = dram.tile([2, NT], F32, tag="cc_in")
            cc_out = dram.tile([2, NT], F32, tag="cc_out")

            # constants
            ones_f = persist.tile([128, 1], F32, tag="ones_f")
            nc.vector.memset(ones_f[:], 1.0)
            ones = persist.tile([128, 1], F32R, tag="ones")
            nc.vector.tensor_copy(ones[:], ones_f[:])
            ones_bf = persist.tile([128, 1], BF16, tag="ones_bf")
            nc.vector.tensor_copy(ones_bf[:], ones_f[:])
            ident_t = persist.tile([128, 128], F32, tag="ident")
            nc.sync.dma_start(ident_t[:], ident[:])

            bq_t = persist.tile([128, CT], F32, tag="bq")
            nc.sync.dma_start(bq_t[:], bq[:])
            bk_t = persist.tile([128, CT], F32, tag="bk")
            nc.sync.dma_start(bk_t[:], bk[:])
            wqn_t = persist.tile([128, CT], F32, tag="wqn")
            nc.sync.dma_start(wqn_t[:], wqn[:])
            wkn_t = persist.tile([128, CT], F32, tag="wkn")
            nc.sync.dma_start(wkn_t[:], wkn[:])

            # pre-reserved pool for the b=0 norm-factor tiles (see baseline)
            from contextlib import ExitStack as _ES
            _rb_ctx = _ES()
            rb0p = _rb_ctx.enter_context(tc.tile_pool(name="rb0p", bufs=1))

            # ---------------- phase 1: qkv projections + ssq partials -------
            with (
                tc.tile_pool(name="wp", bufs=1) as wp,
                tc.tile_pool(name="hp", bufs=2) as hp,
                tc.tile_pool(name="ev", bufs=1) as evp,
            ):
                def load_hall(tb):
                    hall = hp.tile([128, DC, 2, TBS], F8, tag="hall")
                    for cs in range(NCS):
                        nc.sync.dma_start(
                            hall[:, cs * CSUB:(cs + 1) * CSUB, :, :],
                            h4[:, cs * CSUB:(cs + 1) * CSUB, :,
                               tb * TBS:(tb + 1) * TBS])
                    return hall

                def load_w(wdram, tag, order="ct", interleave_tb0=False):
                    wall = wp.tile([128, DC, 2, CW], F8, tag=tag)
                    w4 = wdram.rearrange("(c j p) n -> p c j n", p=128, j=2)
                    hall0 = None
                    if interleave_tb0:
                        hall0 = hp.tile([128, DC, 2, TBS], F8, tag="hall")
                    if order == "ct":
                        wslices = [(slice(None),
                                    slice(ct * 128, (ct + 1) * 128))
                                   for ct in range(CT)]
                    else:
                        wslices = [(slice(cs * CSUB, (cs + 1) * CSUB),
                                    slice(None)) for cs in range(NCS)]
                    for si, (csl, nsl) in enumerate(wslices):
                        nc.sync.dma_start(wall[:, csl, :, nsl],
                                          w4[:, csl, :, nsl])
                        if hall0 is not None and si < NCS:
                            nc.sync.dma_start(
                                hall0[:, si * CSUB:(si + 1) * CSUB, :, :],
                                h4[:, si * CSUB:(si + 1) * CSUB, :, 0:TBS])
                    if hall0 is not None:
                        for cs in range(len(wslices), NCS):
                            nc.sync.dma_start(
                                hall0[:, cs * CSUB:(cs + 1) * CSUB, :, :],
                                h4[:, cs * CSUB:(cs + 1) * CSUB, :, 0:TBS])
                    return wall, hall0

                # ---- q+k in one streamed pass over hT
                with (
                    tc.tile_pool(name="psqk", bufs=4, space="PSUM") as psp,
                    tc.tile_pool(name="sqk", bufs=1, space="PSUM") as sqp,
                    tc.tile_pool(name="aux", bufs=1) as st1,
                ):
                    wall_q, hall0 = load_w(wq8, "wq", interleave_tb0=True)
                    wall_k, _ = load_w(wk8, "wk")
                    walls = [wall_q, wall_k]
                    biases = [bq_t, bk_t]
                    spills = [qsc, ksc]
                    # per-token sum-of-squares accumulators, one full PSUM
                    # bank each ([:, 0:2*NTB] used); one group per tb
                    ssq_all = [sqp.tile([128, 512], F32, tag=f"ssq{gi}",
                                        name=f"ssq{gi}")
                               for gi in range(2)]
                    deferred = []
                    for tb in range(NTB):
                        hall = hall0 if tb == 0 else load_hall(tb)
                        hall0 = None
                        for gi in range(2):
                            wall = walls[gi]
                            evq = evp.tile([128, CT, TBS], BF16, tag="evq",
                                           name="evq")
                            sqt = st1.tile([128, CT, TBS], BF16, tag="sqt",
                                           name=f"sqt{gi}")
                            for ct in range(CT):
                                pq = psp.tile([128, TBS], F32, tag="acc")
                                csl = slice(ct * 128, (ct + 1) * 128)
                                for cp in range(DC // 2):
                                    nc.tensor.matmul(
                                        pq[:],
                                        wall[:, 2 * cp:2 * cp + 2, 0, csl],
                                        hall[:, 2 * cp:2 * cp + 2, 1, :],
                                        start=(cp == 0), stop=False,
                                        perf_mode=DRM)
                                for ch in range(DC):
                                    nc.tensor.matmul(
                                        pq[:],
                                        wall[:, ch, :, csl],
                                        hall[:, ch, :, :],
                                        start=False, stop=(ch == DC - 1),
                                        perf_mode=DRM)
                                if deferred:
                                    deferred.pop(0)()
                                # evq = (pq + bias*scale) * (1/scale)
                                nc.vector.tensor_scalar(
                                    evq[:, ct, :], pq[:],
                                    biases[gi][:, ct:ct + 1], QKV_DESC,
                                    mybir.AluOpType.add,
                                    mybir.AluOpType.mult)
                                nc.sync.dma_start(
                                    spills[gi]
                                    .rearrange("(c p) t -> p c t", p=128)
                                    [:, ct, tb * TBS:(tb + 1) * TBS],
                                    evq[:, ct, :])
                                nc.vector.tensor_mul(
                                    sqt[:, ct, :], evq[:, ct, :],
                                    evq[:, ct, :])

                                def emit_ssq(gi=gi, tb=tb, ct=ct, sqt=sqt):
                                    for c in range(TBS // 128):
                                        nc.tensor.matmul(
                                            ssq_all[gi][:,
                                                        tb * 2 + c:
                                                        tb * 2 + c + 1],
                                            sqt[:, ct,
                                                c * 128:(c + 1) * 128],
                                            ones_bf[:],
                                            start=(ct == 0 and c == 0),
                                            stop=(ct == CT - 1 and c == 1))
                                deferred.append(emit_ssq)
                    while deferred:
                        deferred.pop(0)()
                    # evict ssq: transpose token-partition layout back to
                    # row layout for the allreduce
                    for gi in range(2):
                        stg = st1.tile([128, 2 * NTB], F32, tag="stg",
                                       name=f"stg{gi}")
                        nc.vector.tensor_copy(stg[:],
                                              ssq_all[gi][:, 0:2 * NTB])
                        tst = sqp.tile([2 * NTB, 128], F32, tag="tst",
                                       name=f"tst{gi}")
                        nc.tensor.matmul(tst[:], stg[:], ident_t[:],
                                         is_transpose=True)
                        t32 = st1.tile([2 * NTB, 128], F32, tag="t32",
                                       name=f"t32{gi}")
                        nc.vector.tensor_copy(t32[:], tst[:])
                        nc.gpsimd.dma_start(
                            cc_in[gi:gi + 1, :]
                            .rearrange("o (p t) -> (o p) t", p=2 * NTB),
                            t32[:])

                    # allreduce the ssq partials (overlaps with the v group)
                    if collective:
                        nc.gpsimd.collective_compute(
                            "AllReduce", mybir.AluOpType.add,
                            replica_groups=[list(range(NCORES))],
                            ins=[cc_in[:].opt()], outs=[cc_out[:].opt()])
                    else:
                        # single-core timing-sim variant (TimelineSim has no
                        # collectives); equivalent-size DMA stand-in
                        nc.sync.dma_start(cc_out[:], cc_in[:])

                    # rope tables + b=0 norm factors, in the pre-reserved
                    # pool so their compute overlaps the v group
                    cosT_t = rb0p.tile([128, S], BF16, tag="cosT")
                    nc.sync.dma_start(cosT_t[:], cosT[:])
                    sinrT_t = rb0p.tile([128, S], BF16, tag="sinrT")
                    nc.sync.dma_start(sinrT_t[:], sinrT[:])

                    rb = {}

                    def emit_rb(b, pool, tmp_pool):
                        for gi in range(2):
                            sc1 = (HD / DIM) if gi == 0 else (1.0 / DIM)
                            sc2 = (HD * EPS) if gi == 0 else EPS
                            row = tmp_pool.tile([1, S], F32, tag="rrow",
                                                name="rrow")
                            nc.sync.dma_start(
                                row[:],
                                cc_out[gi:gi + 1, b * S:(b + 1) * S])
                            nc.vector.tensor_scalar(
                                row[:], row[:], sc1, sc2,
                                mybir.AluOpType.mult, mybir.AluOpType.add)
                            nc.scalar.activation(
                                row[:], row[:],
                                mybir.ActivationFunctionType.Sqrt)
                            rowb = tmp_pool.tile([1, S], BF16, tag="rowb",
                                                 name="rowb")
                            with nc.allow_low_precision(
                                    reason="rinv factors are fine in bf16"):
                                nc.vector.reciprocal(rowb[:], row[:])
                            t = pool.tile([128, S], BF16, tag=f"rb{gi}{b}",
                                          name=f"rb{gi}{b}")
                            nc.gpsimd.partition_broadcast(t[:], rowb[:])
                            rb[(gi, b)] = t

                    emit_rb(0, rb0p, st1)


                # ---- v projection (natural layout, h token-tiles stationary)
                VNB = [512, 128] if CW == 640 else [CW]
                with tc.tile_pool(name="psv", bufs=2, space="PSUM") as psv:
                    wall, hall0 = load_w(wv8, "wq", order="cs",
                                         interleave_tb0=True)
                    for tb in range(NTB):
                        hall = hall0 if tb == 0 else load_hall(tb)
                        hall0 = None
                        nsub = TBS // 128
                        pv = [[psv.tile([128, nb], F32, tag=f"pv{ts}_{i}",
                                        name=f"pv{ts}_{i}")
                               for i, nb in enumerate(VNB)]
                              for ts in range(nsub)]
                        for ts in range(nsub):
                            tsl = slice(ts * 128, (ts + 1) * 128)
                            off = 0
                            for i, nb in enumerate(VNB):
                                nsl = slice(off, off + nb)
                                for cp in range(DC // 2):
                                    nc.tensor.matmul(
                                        pv[ts][i][:],
                                        hall[:, 2 * cp:2 * cp + 2, 1, tsl],
                                        wall[:, 2 * cp:2 * cp + 2, 0, nsl],
                                        start=(cp == 0), stop=False,
                                        perf_mode=DRM)
                                for ch in range(DC):
                                    nc.tensor.matmul(
                                        pv[ts][i][:],
                                        hall[:, ch, :, tsl],
                                        wall[:, ch, :, nsl],
                                        start=False, stop=(ch == DC - 1),
                                        perf_mode=DRM)
                                off += nb
                        evv = evp.tile([128, nsub, CW], BF16, tag="evq",
                                       name="evv")
                        for ts in range(nsub):
                            off = 0
                            for i, nb in enumerate(VNB):
                                # v bias folded into the host-side output
                                # bias; descale on the idle scalar engine
                                nc.scalar.activation(
                                    evv[:, ts, off:off + nb],
                                    pv[ts][i][:],
                                    mybir.ActivationFunctionType.Copy,
                                    scale=QKV_DESC)
                                off += nb
                            nc.sync.dma_start(
                                vsc.rearrange("(b p) n -> p b n", p=128)
                                [:, tb * nsub + ts, :],
                                evv[:, ts, :])

            # ---------------- phase 3: attention per (batch, head) ----------
            # wo8 prefix prefetched into its own pool while attention runs
            WOSL = 1024
            w4o = wo8.rearrange("(s j p) n -> p s j n", p=128, j=2)
            with tc.tile_pool(name="wo0p", bufs=1) as wo0p:
                with (
                  tc.tile_pool(name="p3", bufs=2) as p3,
                  tc.tile_pool(name="p3e", bufs=3) as p3e,
                  tc.tile_pool(name="cs3", bufs=1) as cs3,
                  tc.tile_pool(name="ps_sc", bufs=2, space="PSUM") as ps_sc,
                  tc.tile_pool(name="ps_cs", bufs=1, space="PSUM") as ps_cs,
                  tc.tile_pool(name="ps_av", bufs=2, space="PSUM") as ps_av,
                  tc.tile_pool(name="p4", bufs=4) as p4,
                  tc.tile_pool(name="ps4", bufs=1, space="PSUM") as ps4,
                ):
                  wot0 = wo0p.tile([128, 6, 2, DIM], F8, tag="wot0")

                  bhs = [(b, hh) for b in range(2) for hh in range(HPC)]

                  def prep(i, nchunk=1):
                      """Load + norm + rope q/k and load v for pair i."""
                      b, hh = bhs[i]
                      CS2 = S // nchunk
                      qkr = []
                      for gi, (spill, wn) in enumerate(
                              [(qsc, wqn_t), (ksc, wkn_t)]):
                          xt = p3.tile([128, S], BF16, tag="xt", name="xt")
                          tmc = p3.tile([128, S], BF16, tag="tmc",
                                        name="tmc")
                          tms = p3.tile([128, S], BF16, tag="tms",
                                        name="tms")
                          xr = p3.tile([128, S], BF16, tag="xr", name="xr")
                          for cc in range(nchunk):
                              sl = slice(cc * CS2, (cc + 1) * CS2)
                              nc.sync.dma_start(
                                  xt[:, sl],
                                  spill[hh * 128:(hh + 1) * 128,
                                        b * S + cc * CS2:
                                        b * S + (cc + 1) * CS2])
                              nc.vector.tensor_mul(xt[:, sl], xt[:, sl],
                                                   rb[(gi, b)][:, sl])
                              nc.vector.tensor_scalar_mul(
                                  xt[:, sl], xt[:, sl], wn[:, hh:hh + 1])
                              nc.vector.tensor_mul(tmc[:, sl], xt[:, sl],
                                                   cosT_t[:, sl])
                              nc.vector.tensor_mul(
                                  tms[0:64, sl], xt[64:128, sl],
                                  sinrT_t[64:128, sl])
                              nc.vector.tensor_mul(
                                  tms[64:128, sl], xt[0:64, sl],
                                  sinrT_t[0:64, sl])
                              nc.vector.tensor_add(xr[:, sl], tmc[:, sl],
                                                   tms[:, sl])
                          qkr.append(xr)
                      vt = p3.tile([128, NST, 128], BF16, tag="vt",
                                   name="vt")
                      nc.gpsimd.dma_start(
                          vt[:], vsc[b * S:(b + 1) * S,
                                     hh * 128:(hh + 1) * 128]
                          .rearrange("(c p) d -> p c d", p=128))
                      return qkr[0], qkr[1], vt

                  # ---- phase 4 emitter: 512-col blocks of the partial
                  # output projection, interleaved between the score groups
                  # of the b=1 attention pairs (which are exp/ACT-bound) so
                  # the PE always has independent work queued
                  ao4 = aosc8.rearrange("(c j p) t -> p c j t", p=128, j=2)
                  p4_aots = {}

                  def load_aot(tt):
                      aot = p4.tile([128, HPC, 2, 128], F8, tag="aot",
                                    name="aot")
                      nc.sync.dma_start(
                          aot[:], ao4[:, :, :, tt * 128:(tt + 1) * 128])
                      return aot

                  def p4_nb(tt, nb):
                      aot = p4_aots[tt]
                      wsl = wot0[:, :, :, nb * 512:(nb + 1) * 512]
                      po = ps4.tile([128, 512], F32, tag="po")
                      # 8 DoubleRow passes: 2 hi*hi chunk-pairs, 5 crosses,
                      # 1 fused hi*hi+lo*lo for the odd chunk
                      nc.tensor.matmul(
                          po[:], aot[:, 0:2, 1, :], wsl[:, 0:2, 0, :],
                          start=True, stop=False, perf_mode=DRM)
                      nc.tensor.matmul(
                          po[:], aot[:, 2:4, 1, :], wsl[:, 2:4, 0, :],
                          start=False, stop=False, perf_mode=DRM)
                      for ch in range(HPC):
                          nc.tensor.matmul(
                              po[:], aot[:, ch, :, :], wsl[:, ch, :, :],
                              start=False, stop=False, perf_mode=DRM)
                      nc.tensor.matmul(
                          po[:], aot[:, 4, :, :], wsl[:, 5, :, :],
                          start=False, stop=True, perf_mode=DRM)
                      ob = p4.tile([128, 512], BF16, tag="ob")
                      nc.vector.tensor_scalar_mul(ob[:], po[:], OUT_DESC)
                      nc.sync.dma_start(
                          outp[tt * 128:(tt + 1) * 128,
                               nb * 512:(nb + 1) * 512], ob[:])

                  P4B0 = [(tt, nb) for tt in range(NT // 256)
                          for nb in range(ONB)]
                  p4i = [0]

                  def p4_filler():
                      k = p4i[0]
                      if k >= len(P4B0):
                          return
                      tt, nb = P4B0[k]
                      if nb == 0:
                          if tt not in p4_aots:
                              p4_aots[tt] = load_aot(tt)
                          if tt + 1 < NT // 256 and tt + 1 not in p4_aots:
                              p4_aots[tt + 1] = load_aot(tt + 1)
                      p4_nb(tt, nb)
                      p4i[0] += 1
                  NST2 = NST // 2
                  preps = {0: prep(0, nchunk=4)}
                  for i in range(len(bhs)):
                      b, hh = bhs[i]
                      if i + 1 < len(bhs) and i > 0:
                          preps[i + 1] = prep(i + 1)
                      qr, kr, vt = preps.pop(i)
                      aoh8 = p3.tile([128, S], F8, tag="aoh8", name="aoh8")
                      aol8 = p3.tile([128, S], F8, tag="aol8", name="aol8")
                      for sqb in range(NSQ):
                          sqsl = slice(sqb * SQB, (sqb + 1) * SQB)
                          cs4b = ps_cs.tile([128, 512], F32, tag="cs4")
                          av = ps_av.tile([128, SQB], F32, tag="av")
                          # one-group colsum + one-step lookahead: the score
                          # matmuls/exp for group g+1 issue before the
                          # cs/av matmuls of group g, hiding exp latency
                          ets = {}
                          for g in range(NST2 + 1):
                              if g < NST2:
                                  sc = ps_sc.tile([128, 2, SQB], F32,
                                                  tag="sc")
                                  for j in range(2):
                                      st = 2 * g + j
                                      nc.tensor.matmul(
                                          sc[:, j, :],
                                          kr[:, st * 128:(st + 1) * 128],
                                          qr[:, sqsl],
                                          start=True, stop=True)
                                  et = p3e.tile([128, 2, SQB], BF16,
                                                tag="et")
                                  nc.scalar.activation(
                                      et[:], sc[:],
                                      mybir.ActivationFunctionType.Exp)
                                  ets[g] = et
                              if g >= 1:
                                  pg = g - 1
                                  et = ets.pop(pg)
                                  for j in range(2):
                                      st = 2 * pg + j
                                      for c in range(4):
                                          nc.tensor.matmul(
                                              cs4b[:, c:c + 1],
                                              et[:, j,
                                                 c * 128:(c + 1) * 128],
                                              ones_bf[:],
                                              start=(st == 0 and c == 0),
                                              stop=(st == NST - 1
                                                    and c == 3))
                                      nc.tensor.matmul(
                                          av[:], vt[:, st, :], et[:, j, :],
                                          start=(st == 0),
                                          stop=(st == NST - 1))
                                      if b == 1 and j == 1:
                                          p4_filler()
                          # denominator: [sq,1] colsum chunks -> one
                          # [1,512] psum row via 4 single-column transposes
                          # (partition-0 aligned reads only)
                          scs = p3.tile([128, 4], F32, tag="scs")
                          nc.vector.tensor_copy(scs[:], cs4b[:, 0:4])
                          # ONE transpose (start_tensor_calc zeroes the whole
                          # psum region, so multiple transposes into one
                          # region would clobber each other on hardware),
                          # then a tiny psum->sbuf DMA to fold the 4
                          # partition rows into one [1,512] row
                          tc4 = cs4b[0:4, 128:256]
                          nc.tensor.matmul(tc4, scs[:], ident_t[:],
                                           is_transpose=True)
                          s4 = p3.tile([4, 128], F32, tag="s4")
                          nc.vector.tensor_copy(s4[:], tc4)
                          rrow = p3.tile([1, SQB], F32, tag="rrow2")
                          nc.gpsimd.dma_start(
                              rrow[:].rearrange("o (p t) -> (o p) t", p=4),
                              s4[:])
                          nc.vector.reciprocal(rrow[:], rrow[:])
                          nc.vector.tensor_scalar_mul(rrow[:], rrow[:], SA)
                          rb2 = p3.tile([128, SQB], F32, tag="rb2")
                          nc.gpsimd.partition_broadcast(rb2[:], rrow[:])
                          ao32 = p3.tile([128, SQB], BF16, tag="ao32")
                          nc.vector.tensor_mul(ao32[:], av[:], rb2[:])
                          nc.vector.tensor_copy(aoh8[:, sqsl], ao32[:])
                          nc.vector.tensor_tensor(
                              aol8[:, sqsl], ao32[:], aoh8[:, sqsl],
                              mybir.AluOpType.subtract)

                          if i == 0 and sqb == 0:
                              # off the critical path: b=1 norm factors,
                              # second pair's prep, full wo8 prefetch
                              emit_rb(1, cs3, p3)
                              preps[1] = prep(1)
                              for nb in range(ONB):
                                  nc.sync.dma_start(
                                      wot0[:, :, :,
                                           nb * 512:(nb + 1) * 512],
                                      w4o[:, :, :, nb * 512:(nb + 1) * 512])
                      nc.gpsimd.dma_start(
                          aosc8[hh * 256 + 128:hh * 256 + 256,
                                b * S:(b + 1) * S], aoh8[:])
                      nc.gpsimd.dma_start(
                          aosc8[hh * 256:hh * 256 + 128,
                                b * S:(b + 1) * S], aol8[:])

                  # -------- phase 4 drain: remaining token tiles ------
                  while p4i[0] < len(P4B0):
                      p4_filler()
                  for tt in range(NT // 256, NT // 128):
                      if tt not in p4_aots:
                          p4_aots[tt] = load_aot(tt)
                      if tt + 1 < NT // 128 and tt + 1 not in p4_aots:
                          p4_aots[tt + 1] = load_aot(tt + 1)
                      for nb in range(ONB):
                          p4_nb(tt, nb)
            _rb_ctx.close()
    nc.finalize()
    return nc


_PROGRAM_CACHE = {}


def _get_program(S, DIM, H):
    key = (S, DIM, H)
    if key not in _PROGRAM_CACHE:
        _PROGRAM_CACHE[key] = build_program(S, DIM, H)
    return _PROGRAM_CACHE[key]


def _split8(x, scale):
    """fp8e4 hi/lo split of x*scale (hi = rne quant, lo = quantized residual).
    Returns (hi, lo) as fp8 arrays."""
    xs = (np.asarray(x, np.float32) * np.float32(scale))
    hi = xs.astype(E4NP)
    lo = (xs - hi.astype(np.float32)).astype(E4NP)
    return hi, lo


def make_in_maps(S, DIM, H, hidden_cond, hidden_uncond, cos_freqs, sin_freqs,
                 Wqkv, bqkv, wq_norm, wk_norm, Wout, bout):
    HD = 128
    HPC = H // NCORES
    CW = HPC * HD
    NT = 2 * S
    h = np.concatenate([np.asarray(hidden_cond), np.asarray(hidden_uncond)],
                       axis=0).reshape(NT, DIM)
    hT = np.ascontiguousarray(h.T)
    hh, hl = _split8(hT, SH)
    hT8 = np.empty((DIM // 128, 2, 128, NT), E4NP)
    hT8[:, 0] = hl.reshape(DIM // 128, 128, NT)
    hT8[:, 1] = hh.reshape(DIM // 128, 128, NT)
    hT8 = np.ascontiguousarray(hT8.reshape(DIM * 2, NT))

    cosT = np.ascontiguousarray(np.asarray(cos_freqs).T)
    sinT = np.asarray(sin_freqs).T  # [128, S]
    HF = HD // 2
    sinrT = np.ascontiguousarray(
        np.concatenate([sinT[HF:], -sinT[:HF]], axis=0))
    Wqkv = np.asarray(Wqkv)
    bqkv = np.asarray(bqkv)
    wq_norm = np.asarray(wq_norm)
    wk_norm = np.asarray(wk_norm)
    Wout = np.asarray(Wout)
    identity = np.eye(128, dtype=np.float32)

    def pack_w(Wslice):
        # [DIM, CW] -> [(c p j), CW] with j=(hi, lo)
        wh, wl = _split8(Wslice, SW)
        n = Wslice.shape[1]
        w8 = np.empty((DIM // 128, 2, 128, n), E4NP)
        w8[:, 0] = wh.reshape(DIM // 128, 128, n)
        w8[:, 1] = wl.reshape(DIM // 128, 128, n)
        return np.ascontiguousarray(w8.reshape(DIM * 2, n))

    in_maps = []
    for c in range(NCORES):
        sl = slice(c * CW, (c + 1) * CW)
        bq_c = (bqkv[0 * DIM:1 * DIM][sl] * (SH * SW)).astype(np.float32) \
            .reshape(HPC, HD).T
        bk_c = (bqkv[1 * DIM:2 * DIM][sl] * (SH * SW)).astype(np.float32) \
            .reshape(HPC, HD).T
        # out-proj: rows for this core, scaled, hi/lo; slots 0..4 (hi,lo),
        # slot 5 = chunk 4 as (lo,hi)
        woh, wol = _split8(Wout[sl, :], SWO)   # [CW, DIM] each
        wo8 = np.empty((6, 2, 128, DIM), E4NP)
        for s in range(HPC):
            wo8[s, 0] = woh[s * 128:(s + 1) * 128]
            wo8[s, 1] = wol[s * 128:(s + 1) * 128]
        wo8[5, 0] = wol[4 * 128:5 * 128]
        wo8[5, 1] = woh[4 * 128:5 * 128]
        in_maps.append({
            "hT8": hT8,
            "wq8": pack_w(Wqkv[:, 0 * DIM:1 * DIM][:, sl]),
            "wk8": pack_w(Wqkv[:, 1 * DIM:2 * DIM][:, sl]),
            "wv8": pack_w(Wqkv[:, 2 * DIM:3 * DIM][:, sl]),
            "bq": np.ascontiguousarray(bq_c),
            "bk": np.ascontiguousarray(bk_c),
            "wqn": np.ascontiguousarray(wq_norm[sl].reshape(HPC, HD).T
                                        .astype(np.float32)),
            "wkn": np.ascontiguousarray(wk_norm[sl].reshape(HPC, HD).T
                                        .astype(np.float32)),
            "cosT": cosT.astype(np.float32),
            "sinrT": sinrT.astype(np.float32),
            "ident": identity,
            "wo8": np.ascontiguousarray(wo8.reshape(6 * 2 * 128, DIM)),
        })
    return in_maps


def run(S, DIM, H, inputs):
    nc = _get_program(S, DIM, H)
    in_maps = make_in_maps(S, DIM, H, **inputs)
    res = run_bass_kernel_spmd(nc, in_maps, list(range(NCORES)))
    partial = np.zeros((2 * S, DIM), np.float64)
    for r in res.results:
        partial += np.asarray(r["outp"]).astype(np.float64)
    # v-bias contribution: softmax rows sum to 1, so attn(v + 1*bv) =
    # attn(v) + 1*bv, and bv flows through Wout as a constant term
    bv_full = np.asarray(inputs["bqkv"])[2 * DIM:3 * DIM].astype(np.float64)
    const_bias = bv_full @ np.asarray(inputs["Wout"]).astype(np.float64) \
        + np.asarray(inputs["bout"])
    out = (partial + const_bias[None, :]).astype(np.float32)
    out = out.reshape(2, 1, S, DIM)
    return out[0], out[1]


def kernel(hidden_cond, hidden_uncond, cos_freqs, sin_freqs,
           Wqkv, bqkv, wq_norm, wk_norm, Wout, bout):
    B, S, DIM = np.asarray(hidden_cond).shape
    assert B == 1
    H = DIM // 128
    return run(S, DIM, H, dict(
        hidden_cond=hidden_cond, hidden_uncond=hidden_uncond,
        cos_freqs=cos_freqs, sin_freqs=sin_freqs, Wqkv=Wqkv, bqkv=bqkv,
        wq_norm=wq_norm, wk_norm=wk_norm, Wout=Wout, bout=bout))
